# revision 18
# baseline (speedup 1.0000x reference)
"""Distributed Trainium2 kernel for DGI-GCN message passing.

Computes out = PReLU(A @ (X @ W^T) + bias) with A a sparse COO matrix
(160k edges, rows sorted), X [10000, 512], W [512, 512].

Strategy (8 NeuronCores, SPMD, no collectives):
  - Nodes (rows of A / output) are sharded 1250 per core.  adj_row is
    sorted, so each core owns a contiguous edge range.
  - Reordered as out = (A @ X) @ W^T: aggregate first (each core only
    transforms its own 1250 rows afterwards), so the fc matmul is tiny.
  - X is replicated (bf16) in every core's DRAM; each core dma_gathers
    the source rows of its edges (bf16, 1KB/row) via 4 SWDGE queues.
  - The segment-sum over sorted rows is a TensorE matmul per 128-node
    tile: psum[nodes,feat] += Bt[edges,nodes].T @ G[edges,feat] where
    Bt is a host-built one-hot(edge->local row) * adj_val matrix.
  - agg is transposed on TensorE (identity matmul) to put features on
    partitions, then multiplied by W^T; bias is added with a K=1
    matmul of ones x bias; PReLU runs on VectorE with the slope baked
    in as an immediate.
"""

import numpy as np
import ml_dtypes

N = 10000          # nodes
F = 512            # in features
H = 512            # hidden (== F)
NCORES = 8
NPC = N // NCORES  # nodes per core (1250)
P = 128
NT = (NPC + P - 1) // P   # node tiles per core (10; last tile has 98 rows)
KT = F // P               # feature chunks (4)
GC = 4                    # chunks per dma_gather call (512 idxs, half a SWDGE ring)
NQ = 4                    # SWDGE queues
SCRATCH = 65536           # dynamic DMA descriptor carveout (bytes/partition)

bf16 = ml_dtypes.bfloat16

_prog_cache = {}


def _build_program(C, alpha, bias_nonzero):
    import concourse.bacc as bacc
    import concourse.tile as tile
    import concourse.mybir as mybir

    dt = mybir.dt
    W_IDX = C * P // 16   # idx columns per node tile

    nc = bacc.Bacc("TRN2", num_swdge_queues=NQ, dynamic_dma_scratch_size=SCRATCH)
    xb = nc.dram_tensor("xb", [N, F], dt.bfloat16, kind="ExternalInput")
    gidx = nc.dram_tensor("gidx", [P, NT * W_IDX], dt.int16, kind="ExternalInput")
    btv = nc.dram_tensor("btv", [P, NT * C, P], dt.bfloat16, kind="ExternalInput")
    wtb = nc.dram_tensor("wtb", [P, KT, H], dt.bfloat16, kind="ExternalInput")
    idn = nc.dram_tensor("idn", [P, P], dt.bfloat16, kind="ExternalInput")
    onesb = nc.dram_tensor("onesb", [1, P], dt.bfloat16, kind="ExternalInput")
    biasb = nc.dram_tensor("biasb", [1, H], dt.bfloat16, kind="ExternalInput")
    outd = nc.dram_tensor("out", [NPC, H], dt.float32, kind="ExternalOutput")

    qn = [0]

    with tile.TileContext(nc) as tc:
        with (
            tc.tile_pool(name="const", bufs=1) as cp,
            tc.tile_pool(name="btp", bufs=NT) as btp,
            tc.tile_pool(name="gp", bufs=12) as gp,
            tc.tile_pool(name="aggps", bufs=2, space="PSUM") as aggps,
            tc.tile_pool(name="trps", bufs=2, space="PSUM") as trps,
            tc.tile_pool(name="outps", bufs=2, space="PSUM") as outps,
            tc.tile_pool(name="sbp", bufs=2) as sbp,
        ):
            idx_t = cp.tile([P, NT * W_IDX], dt.int16)
            nc.sync.dma_start(out=idx_t[:], in_=gidx[:])
            id_t = cp.tile([P, P], dt.bfloat16)
            nc.sync.dma_start(out=id_t[:], in_=idn[:])

            # warmup 1: tiny self-contained dma_gather pays the Q7 ext-isa
            # IRAM load (~6us) while the real index tile is still in flight
            nidx_regs = {}
            c0 = 0
            while c0 < C:
                ch = min(GC, C - c0)
                if ch * P not in nidx_regs:
                    nidx_regs[ch * P] = nc.gpsimd.to_reg(ch * P)
                c0 += ch
            if P not in nidx_regs:
                nidx_regs[P] = nc.gpsimd.to_reg(P)
            # warmup 2: dummy transposes heat the TensorE HAM clock gate so
            # the first real matmuls run at 2.4 GHz
            warm_ps = trps.tile([P, P], dt.bfloat16, space="PSUM", tag="trp")
            for _ in range(18):
                nc.tensor.transpose(out=warm_ps[:], in_=id_t[:], identity=id_t[:])

            wt_t = cp.tile([P, KT, H], dt.bfloat16)
            nc.sync.dma_start(out=wt_t[:], in_=wtb[:])
            ones_t = cp.tile([1, P], dt.bfloat16)
            nc.sync.dma_start(out=ones_t[:], in_=onesb[:])
            bias_t = cp.tile([1, H], dt.bfloat16)
            nc.sync.dma_start(out=bias_t[:], in_=biasb[:])

            bts = []
            for t in range(NT):
                bt_t = btp.tile([P, C, P], dt.bfloat16, tag="bt")
                nc.sync.dma_start(out=bt_t[:], in_=btv[:, t * C:(t + 1) * C, :])
                bts.append(bt_t)

            def issue_gathers(t):
                gcalls = []
                c0 = 0
                while c0 < C:
                    ch = min(GC, C - c0)
                    g = gp.tile([P, GC, F], dt.bfloat16, tag="g")
                    nidx = ch * P
                    w0 = t * W_IDX + c0 * (P // 16)
                    nc.gpsimd.dma_gather(
                        g[:, :ch, :], xb[:], idx_t[:, w0:w0 + nidx // 16],
                        nidx, nidx_regs[nidx], F, queue_num=qn[0] % NQ)
                    qn[0] += 1
                    gcalls.append((g, c0, ch))
                    c0 += ch
                return gcalls

            # all gathers issued up-front at max priority: keeps the 4 SWDGE
            # queues saturated for the whole kernel
            gcalls_per_tile = [issue_gathers(t) for t in range(NT)]

            for t in range(NT):
                gcalls = gcalls_per_tile[t]
                agg = aggps.tile([P, F], dt.float32, space="PSUM", tag="agg")
                for g, c0, ch in gcalls:
                    for j in range(ch):
                        c = c0 + j
                        nc.tensor.matmul(
                            agg[:], lhsT=bts[t][:, c, :], rhs=g[:, j, :],
                            start=(c == 0), stop=(c == C - 1))

                agg_sb = sbp.tile([P, F], dt.bfloat16, tag="aggsb")
                nc.vector.tensor_copy(out=agg_sb[:], in_=agg[:])

                aggT = sbp.tile([P, KT, P], dt.bfloat16, tag="aggT")
                for k in range(KT):
                    tp = trps.tile([P, P], dt.bfloat16, space="PSUM", tag="trp")
                    nc.tensor.transpose(
                        out=tp[:], in_=agg_sb[:, k * P:(k + 1) * P], identity=id_t[:])
                    nc.vector.tensor_copy(out=aggT[:, k, :], in_=tp[:])

                ops_ = outps.tile([P, H], dt.float32, space="PSUM", tag="ops")
                for k in range(KT):
                    nc.tensor.matmul(
                        ops_[:], lhsT=aggT[:, k, :], rhs=wt_t[:, k, :],
                        start=(k == 0), stop=(k == KT - 1 and not bias_nonzero))
                if bias_nonzero:
                    nc.tensor.matmul(
                        ops_[:], lhsT=ones_t[:1, :], rhs=bias_t[:1, :],
                        start=False, stop=True)

                pos = sbp.tile([P, H], dt.float32, tag="pos")
                if 0.0 <= alpha <= 1.0:
                    # PReLU(z) = max(alpha*z, z) for slope in [0, 1]
                    zsb = sbp.tile([P, H], dt.float32, tag="zsb")
                    nc.vector.tensor_copy(out=zsb[:], in_=ops_[:])
                    nc.vector.scalar_tensor_tensor(
                        out=pos[:], in0=ops_[:], scalar=float(alpha), in1=zsb[:],
                        op0=mybir.AluOpType.mult, op1=mybir.AluOpType.max)
                else:
                    neg = sbp.tile([P, H], dt.float32, tag="neg")
                    nc.vector.tensor_scalar_max(out=pos[:], in0=ops_[:], scalar1=0.0)
                    nc.vector.tensor_scalar(
                        out=neg[:], in0=ops_[:], scalar1=0.0, scalar2=float(alpha),
                        op0=mybir.AluOpType.min, op1=mybir.AluOpType.mult)
                    nc.vector.tensor_add(out=pos[:], in0=pos[:], in1=neg[:])

                valid = min(P, NPC - t * P)
                nc.sync.dma_start(out=outd[t * P:t * P + valid, :], in_=pos[:valid, :])

    nc.compile()
    return nc


def kernel(**inputs):
    from concourse.bass_utils import run_bass_kernel_spmd

    x = np.asarray(inputs["x"], dtype=np.float32)
    fc_w = np.asarray(inputs["fc_w"], dtype=np.float32)
    bias = np.asarray(inputs["bias"], dtype=np.float32).reshape(-1)
    alpha = float(np.asarray(inputs["prelu_a"]).reshape(-1)[0])
    adj_vals = np.asarray(inputs["adj_vals"], dtype=np.float32)
    adj_row = np.asarray(inputs["adj_row"]).astype(np.int64)
    adj_col = np.asarray(inputs["adj_col"]).astype(np.int64)

    # --- host-side 1D graph partition (nodes -> cores, tiles of 128) ---
    starts = []
    for i in range(NCORES):
        for t in range(NT):
            starts.append(i * NPC + min(t * P, NPC))
    starts.append(N)
    bounds = np.searchsorted(adj_row, np.asarray(starts))

    # Per (core, tile): deduplicate source columns (one gather per distinct
    # source, Bt column carries all its in-tile edges) and keep them sorted
    # for HBM locality.  tiles[j] = (uniq_cols, u_pos_per_edge, rows, vals)
    tiles = []
    n_uniq = np.zeros(NCORES * NT, np.int64)
    for j in range(NCORES * NT):
        lo, hi = int(bounds[j]), int(bounds[j + 1])
        cols = adj_col[lo:hi]
        uniq, inv = np.unique(cols, return_inverse=True)
        rl = adj_row[lo:hi] - starts[j]
        tiles.append((uniq, inv, rl, adj_vals[lo:hi]))
        n_uniq[j] = len(uniq)
    C = max(1, int(np.max((n_uniq + P - 1) // P)))  # uniform chunk budget per tile

    bias_nonzero = bool(np.any(bias != 0.0))
    key = (C, np.float32(alpha).tobytes(), bias_nonzero)
    if key not in _prog_cache:
        _prog_cache[key] = _build_program(C, alpha, bias_nonzero)
    nc = _prog_cache[key]

    W_IDX = C * P // 16
    xb = np.ascontiguousarray(x.astype(bf16))
    wtb = np.ascontiguousarray(fc_w.T.reshape(KT, P, H).transpose(1, 0, 2).astype(bf16))
    idn = np.eye(P, dtype=bf16)
    onesb = np.ones((1, P), dtype=bf16)
    biasb = bias.reshape(1, H).astype(bf16)

    in_maps = []
    for i in range(NCORES):
        gidx_flat = np.zeros(NT * C * P, np.int16)
        btv_f = np.zeros((P, NT * C, P), np.float32)
        for t in range(NT):
            j = i * NT + t
            uniq, inv, rl, vals = tiles[j]
            nu = len(uniq)
            if nu == 0:
                continue
            gidx_flat[t * C * P:t * C * P + nu] = uniq.astype(np.int16)
            np.add.at(btv_f, (inv % P, t * C + inv // P, rl), vals)
        # wrap indices: per GC-chunk gather call, column-major over 16 rows
        idx_host = np.zeros((P, NT * W_IDX), np.int16)
        for t in range(NT):
            c0 = 0
            while c0 < C:
                ch = min(GC, C - c0)
                nidx = ch * P
                seg = gidx_flat[t * C * P + c0 * P: t * C * P + c0 * P + nidx]
                w0 = t * W_IDX + c0 * (P // 16)
                idx_host[:, w0:w0 + nidx // 16] = np.tile(
                    seg.reshape(nidx // 16, 16).T, (8, 1))
                c0 += ch
        in_maps.append({
            "xb": xb, "gidx": idx_host, "btv": btv_f.astype(bf16),
            "wtb": wtb, "idn": idn, "onesb": onesb, "biasb": biasb,
        })

    res = run_bass_kernel_spmd(nc, in_maps, core_ids=list(range(NCORES)))
    out = np.concatenate([res.results[i]["out"] for i in range(NCORES)], axis=0)
    return np.ascontiguousarray(out.astype(np.float32))


# revision 19
# speedup vs baseline: 1.0518x; 1.0518x over previous
"""Distributed Trainium2 kernel for DGI-GCN message passing.

Computes out = PReLU(A @ (X @ W^T) + bias) with A a sparse COO matrix
(160k edges, rows sorted), X [10000, 512], W [512, 512].

Strategy (8 NeuronCores, SPMD, no collectives):
  - Nodes (rows of A / output) are sharded 1250 per core.  adj_row is
    sorted, so each core owns a contiguous edge range.
  - Reordered as out = (A @ X) @ W^T: aggregate first (each core only
    transforms its own 1250 rows afterwards), so the fc matmul is tiny.
  - X is replicated (bf16) in every core's DRAM; each core dma_gathers
    the source rows of its edges (bf16, 1KB/row) via 4 SWDGE queues.
  - The segment-sum over sorted rows is a TensorE matmul per 128-node
    tile: psum[nodes,feat] += Bt[edges,nodes].T @ G[edges,feat] where
    Bt is a host-built one-hot(edge->local row) * adj_val matrix.
  - agg is transposed on TensorE (identity matmul) to put features on
    partitions, then multiplied by W^T; bias is added with a K=1
    matmul of ones x bias; PReLU runs on VectorE with the slope baked
    in as an immediate.
"""

import numpy as np
import ml_dtypes

N = 10000          # nodes
F = 512            # in features
H = 512            # hidden (== F)
NCORES = 8
NPC = N // NCORES  # nodes per core (1250)
P = 128
NT = (NPC + P - 1) // P   # node tiles per core (10; last tile has 98 rows)
KT = F // P               # feature chunks (4)
GC = 4                    # chunks per dma_gather call (512 idxs, half a SWDGE ring)
NQ = 4                    # SWDGE queues
SCRATCH = 65536           # dynamic DMA descriptor carveout (bytes/partition)

bf16 = ml_dtypes.bfloat16

_prog_cache = {}


def _build_program(C, alpha, bias_nonzero):
    import concourse.bacc as bacc
    import concourse.tile as tile
    import concourse.mybir as mybir

    dt = mybir.dt
    W_IDX = C * P // 16   # idx columns per node tile

    nc = bacc.Bacc("TRN2", num_swdge_queues=NQ, dynamic_dma_scratch_size=SCRATCH)
    xb = nc.dram_tensor("xb", [N, F], dt.bfloat16, kind="ExternalInput")
    gidx = nc.dram_tensor("gidx", [P, NT * W_IDX], dt.int16, kind="ExternalInput")
    btv = nc.dram_tensor("btv", [P, NT * C, P], dt.bfloat16, kind="ExternalInput")
    wtb = nc.dram_tensor("wtb", [P, KT, H], dt.bfloat16, kind="ExternalInput")
    idn = nc.dram_tensor("idn", [P, P], dt.bfloat16, kind="ExternalInput")
    onesb = nc.dram_tensor("onesb", [1, P], dt.bfloat16, kind="ExternalInput")
    biasb = nc.dram_tensor("biasb", [1, H], dt.bfloat16, kind="ExternalInput")
    outd = nc.dram_tensor("out", [NPC, H], dt.float32, kind="ExternalOutput")

    qn = [0]

    with tile.TileContext(nc) as tc:
        with (
            tc.tile_pool(name="const", bufs=1) as cp,
            tc.tile_pool(name="btp", bufs=NT) as btp,
            tc.tile_pool(name="gp", bufs=8) as gp,
            tc.tile_pool(name="aggps", bufs=3, space="PSUM") as aggps,
            tc.tile_pool(name="trps", bufs=2, space="PSUM") as trps,
            tc.tile_pool(name="outps", bufs=2, space="PSUM") as outps,
            tc.tile_pool(name="sbp", bufs=2) as sbp,
        ):
            idx_t = cp.tile([P, NT * W_IDX], dt.int16)
            nc.sync.dma_start(out=idx_t[:], in_=gidx[:])
            id_t = cp.tile([P, P], dt.bfloat16)
            nc.sync.dma_start(out=id_t[:], in_=idn[:])

            # warmup 1: tiny self-contained dma_gather pays the Q7 ext-isa
            # IRAM load (~6us) while the real index tile is still in flight
            nidx_regs = {}
            c0 = 0
            while c0 < C:
                ch = min(GC, C - c0)
                if ch * P not in nidx_regs:
                    nidx_regs[ch * P] = nc.gpsimd.to_reg(ch * P)
                c0 += ch
            if P not in nidx_regs:
                nidx_regs[P] = nc.gpsimd.to_reg(P)
            # warmup 2: dummy transposes heat the TensorE HAM clock gate so
            # the first real matmuls run at 2.4 GHz
            warm_ps = trps.tile([P, P], dt.bfloat16, space="PSUM", tag="trp")
            for _ in range(18):
                nc.tensor.transpose(out=warm_ps[:], in_=id_t[:], identity=id_t[:])

            wt_t = cp.tile([P, KT, H], dt.bfloat16)
            nc.sync.dma_start(out=wt_t[:], in_=wtb[:])
            ones_t = cp.tile([1, P], dt.bfloat16)
            nc.sync.dma_start(out=ones_t[:], in_=onesb[:])
            bias_t = cp.tile([1, H], dt.bfloat16)
            nc.sync.dma_start(out=bias_t[:], in_=biasb[:])

            bts = []
            for t in range(NT):
                bt_t = btp.tile([P, C, P], dt.bfloat16, tag="bt")
                nc.sync.dma_start(out=bt_t[:], in_=btv[:, t * C:(t + 1) * C, :])
                bts.append(bt_t)

            def issue_gathers(t):
                gcalls = []
                c0 = 0
                while c0 < C:
                    ch = min(GC, C - c0)
                    g = gp.tile([P, GC, F], dt.bfloat16, tag="g")
                    nidx = ch * P
                    w0 = t * W_IDX + c0 * (P // 16)
                    nc.gpsimd.dma_gather(
                        g[:, :ch, :], xb[:], idx_t[:, w0:w0 + nidx // 16],
                        nidx, nidx_regs[nidx], F, queue_num=qn[0] % NQ)
                    qn[0] += 1
                    gcalls.append((g, c0, ch))
                    c0 += ch
                return gcalls

            # all gathers issued up-front at max priority: keeps the 4 SWDGE
            # queues saturated for the whole kernel
            gcalls_per_tile = [issue_gathers(t) for t in range(NT)]

            for t in range(NT):
                gcalls = gcalls_per_tile[t]
                agg = aggps.tile([P, F], dt.float32, space="PSUM", tag="agg")
                for g, c0, ch in gcalls:
                    for j in range(ch):
                        c = c0 + j
                        nc.tensor.matmul(
                            agg[:], lhsT=bts[t][:, c, :], rhs=g[:, j, :],
                            start=(c == 0), stop=(c == C - 1))

                agg_sb = sbp.tile([P, F], dt.bfloat16, tag="aggsb")
                nc.vector.tensor_copy(out=agg_sb[:], in_=agg[:])

                aggT = sbp.tile([P, KT, P], dt.bfloat16, tag="aggT")
                for k in range(KT):
                    tp = trps.tile([P, P], dt.bfloat16, space="PSUM", tag="trp")
                    nc.tensor.transpose(
                        out=tp[:], in_=agg_sb[:, k * P:(k + 1) * P], identity=id_t[:])
                    nc.vector.tensor_copy(out=aggT[:, k, :], in_=tp[:])

                ops_ = outps.tile([P, H], dt.float32, space="PSUM", tag="ops")
                for k in range(KT):
                    nc.tensor.matmul(
                        ops_[:], lhsT=aggT[:, k, :], rhs=wt_t[:, k, :],
                        start=(k == 0), stop=(k == KT - 1 and not bias_nonzero))
                if bias_nonzero:
                    nc.tensor.matmul(
                        ops_[:], lhsT=ones_t[:1, :], rhs=bias_t[:1, :],
                        start=False, stop=True)

                pos = sbp.tile([P, H], dt.float32, tag="pos")
                if 0.0 <= alpha <= 1.0:
                    # PReLU(z) = max(alpha*z, z) for slope in [0, 1]
                    zsb = sbp.tile([P, H], dt.float32, tag="zsb")
                    nc.vector.tensor_copy(out=zsb[:], in_=ops_[:])
                    nc.vector.scalar_tensor_tensor(
                        out=pos[:], in0=ops_[:], scalar=float(alpha), in1=zsb[:],
                        op0=mybir.AluOpType.mult, op1=mybir.AluOpType.max)
                else:
                    neg = sbp.tile([P, H], dt.float32, tag="neg")
                    nc.vector.tensor_scalar_max(out=pos[:], in0=ops_[:], scalar1=0.0)
                    nc.vector.tensor_scalar(
                        out=neg[:], in0=ops_[:], scalar1=0.0, scalar2=float(alpha),
                        op0=mybir.AluOpType.min, op1=mybir.AluOpType.mult)
                    nc.vector.tensor_add(out=pos[:], in0=pos[:], in1=neg[:])

                valid = min(P, NPC - t * P)
                nc.sync.dma_start(out=outd[t * P:t * P + valid, :], in_=pos[:valid, :])

    nc.compile()
    return nc


def kernel(**inputs):
    from concourse.bass_utils import run_bass_kernel_spmd

    x = np.asarray(inputs["x"], dtype=np.float32)
    fc_w = np.asarray(inputs["fc_w"], dtype=np.float32)
    bias = np.asarray(inputs["bias"], dtype=np.float32).reshape(-1)
    alpha = float(np.asarray(inputs["prelu_a"]).reshape(-1)[0])
    adj_vals = np.asarray(inputs["adj_vals"], dtype=np.float32)
    adj_row = np.asarray(inputs["adj_row"]).astype(np.int64)
    adj_col = np.asarray(inputs["adj_col"]).astype(np.int64)

    # --- host-side 1D graph partition (nodes -> cores, tiles of 128) ---
    starts = []
    for i in range(NCORES):
        for t in range(NT):
            starts.append(i * NPC + min(t * P, NPC))
    starts.append(N)
    bounds = np.searchsorted(adj_row, np.asarray(starts))

    # Per (core, tile): deduplicate source columns (one gather per distinct
    # source, Bt column carries all its in-tile edges) and keep them sorted
    # for HBM locality.  tiles[j] = (uniq_cols, u_pos_per_edge, rows, vals)
    tiles = []
    n_uniq = np.zeros(NCORES * NT, np.int64)
    for j in range(NCORES * NT):
        lo, hi = int(bounds[j]), int(bounds[j + 1])
        cols = adj_col[lo:hi]
        uniq, inv = np.unique(cols, return_inverse=True)
        rl = adj_row[lo:hi] - starts[j]
        tiles.append((uniq, inv, rl, adj_vals[lo:hi]))
        n_uniq[j] = len(uniq)
    C = max(1, int(np.max((n_uniq + P - 1) // P)))  # uniform chunk budget per tile

    bias_nonzero = bool(np.any(bias != 0.0))
    key = (C, np.float32(alpha).tobytes(), bias_nonzero)
    if key not in _prog_cache:
        _prog_cache[key] = _build_program(C, alpha, bias_nonzero)
    nc = _prog_cache[key]

    W_IDX = C * P // 16
    xb = np.ascontiguousarray(x.astype(bf16))
    wtb = np.ascontiguousarray(fc_w.T.reshape(KT, P, H).transpose(1, 0, 2).astype(bf16))
    idn = np.eye(P, dtype=bf16)
    onesb = np.ones((1, P), dtype=bf16)
    biasb = bias.reshape(1, H).astype(bf16)

    in_maps = []
    for i in range(NCORES):
        gidx_flat = np.zeros(NT * C * P, np.int16)
        btv_f = np.zeros((P, NT * C, P), np.float32)
        for t in range(NT):
            j = i * NT + t
            uniq, inv, rl, vals = tiles[j]
            nu = len(uniq)
            if nu == 0:
                continue
            gidx_flat[t * C * P:t * C * P + nu] = uniq.astype(np.int16)
            np.add.at(btv_f, (inv % P, t * C + inv // P, rl), vals)
        # wrap indices: per GC-chunk gather call, column-major over 16 rows
        idx_host = np.zeros((P, NT * W_IDX), np.int16)
        for t in range(NT):
            c0 = 0
            while c0 < C:
                ch = min(GC, C - c0)
                nidx = ch * P
                seg = gidx_flat[t * C * P + c0 * P: t * C * P + c0 * P + nidx]
                w0 = t * W_IDX + c0 * (P // 16)
                idx_host[:, w0:w0 + nidx // 16] = np.tile(
                    seg.reshape(nidx // 16, 16).T, (8, 1))
                c0 += ch
        in_maps.append({
            "xb": xb, "gidx": idx_host, "btv": btv_f.astype(bf16),
            "wtb": wtb, "idn": idn, "onesb": onesb, "biasb": biasb,
        })

    res = run_bass_kernel_spmd(nc, in_maps, core_ids=list(range(NCORES)))
    out = np.concatenate([res.results[i]["out"] for i in range(NCORES)], axis=0)
    return np.ascontiguousarray(out.astype(np.float32))


# revision 22
# speedup vs baseline: 1.2562x; 1.1944x over previous
"""Distributed Trainium2 kernel for DGI-GCN message passing.

Computes out = PReLU(A @ (X @ W^T) + bias) with A a sparse COO matrix
(160k edges, rows sorted), X [10000, 512], W [512, 512].

Strategy (8 NeuronCores, SPMD, no collectives):
  - Nodes (rows of A / output) are sharded 1250 per core.  adj_row is
    sorted, so each core owns a contiguous edge range.
  - Reordered as out = (A @ X) @ W^T: aggregate first (each core only
    transforms its own 1250 rows afterwards), so the fc matmul is tiny.
  - X is replicated (bf16) in every core's DRAM; each core dma_gathers
    the source rows of its edges (bf16, 1KB/row) via 4 SWDGE queues.
  - The segment-sum over sorted rows is a TensorE matmul per 128-node
    tile: psum[nodes,feat] += Bt[edges,nodes].T @ G[edges,feat] where
    Bt is a host-built one-hot(edge->local row) * adj_val matrix.
  - agg is transposed on TensorE (identity matmul) to put features on
    partitions, then multiplied by W^T; bias is added with a K=1
    matmul of ones x bias; PReLU runs on VectorE with the slope baked
    in as an immediate.
"""

import numpy as np
import ml_dtypes

N = 10000          # nodes
F = 512            # in features
H = 512            # hidden (== F)
NCORES = 8
NPC = N // NCORES  # nodes per core (1250)
P = 128
NT = (NPC + P - 1) // P   # node tiles per core (10; last tile has 98 rows)
KT = F // P               # feature chunks (4)
GC = 4                    # chunks per dma_gather call (512 idxs, half a SWDGE ring)
NQ = 4                    # SWDGE queues
SCRATCH = 65536           # dynamic DMA descriptor carveout (bytes/partition)

bf16 = ml_dtypes.bfloat16

_prog_cache = {}


def _build_program(C, alpha, bias_nonzero):
    import concourse.bacc as bacc
    import concourse.tile as tile
    import concourse.mybir as mybir

    dt = mybir.dt
    W_IDX = C * P // 16   # idx columns per node tile

    nc = bacc.Bacc("TRN2", num_swdge_queues=NQ, dynamic_dma_scratch_size=SCRATCH)
    xb = nc.dram_tensor("xb", [N, F], dt.bfloat16, kind="ExternalInput")
    gidx = nc.dram_tensor("gidx", [P, NT * W_IDX], dt.int16, kind="ExternalInput")
    btv = nc.dram_tensor("btv", [P, NT * C, P], dt.bfloat16, kind="ExternalInput")
    wtb = nc.dram_tensor("wtb", [P, KT, H], dt.bfloat16, kind="ExternalInput")
    idn = nc.dram_tensor("idn", [P, P], dt.bfloat16, kind="ExternalInput")
    onesb = nc.dram_tensor("onesb", [1, P], dt.bfloat16, kind="ExternalInput")
    biasb = nc.dram_tensor("biasb", [1, H], dt.bfloat16, kind="ExternalInput")
    outd = nc.dram_tensor("out", [NPC, H], dt.float32, kind="ExternalOutput")

    qn = [0]

    with tile.TileContext(nc) as tc:
        with (
            tc.tile_pool(name="const", bufs=1) as cp,
            tc.tile_pool(name="btp", bufs=NT) as btp,
            tc.tile_pool(name="gp", bufs=8) as gp,
            tc.tile_pool(name="aggps", bufs=2, space="PSUM") as aggps,
            tc.tile_pool(name="trps", bufs=2, space="PSUM") as trps,
            tc.tile_pool(name="outps", bufs=2, space="PSUM") as outps,
            tc.tile_pool(name="sbp", bufs=2) as sbp,
        ):
            idx_t = cp.tile([P, NT * W_IDX], dt.int16)
            nc.sync.dma_start(out=idx_t[:], in_=gidx[:])
            id_t = cp.tile([P, P], dt.bfloat16)
            nc.sync.dma_start(out=id_t[:], in_=idn[:])

            # warmup 1: tiny self-contained dma_gather pays the Q7 ext-isa
            # IRAM load (~6us) while the real index tile is still in flight
            nidx_regs = {}
            c0 = 0
            while c0 < C:
                ch = min(GC, C - c0)
                if ch * P not in nidx_regs:
                    nidx_regs[ch * P] = nc.gpsimd.to_reg(ch * P)
                c0 += ch
            if P not in nidx_regs:
                nidx_regs[P] = nc.gpsimd.to_reg(P)
            # warmup 2: dummy transposes heat the TensorE HAM clock gate so
            # the first real matmuls run at 2.4 GHz
            warm_ps = trps.tile([P, P], dt.bfloat16, space="PSUM", tag="trp")
            for _ in range(18):
                nc.tensor.transpose(out=warm_ps[:], in_=id_t[:], identity=id_t[:])

            wt_t = cp.tile([P, KT, H], dt.bfloat16)
            nc.sync.dma_start(out=wt_t[:], in_=wtb[:])
            ones_t = cp.tile([1, P], dt.bfloat16)
            nc.sync.dma_start(out=ones_t[:], in_=onesb[:])
            bias_t = cp.tile([1, H], dt.bfloat16)
            nc.sync.dma_start(out=bias_t[:], in_=biasb[:])

            bts = []
            for t in range(NT):
                bt_t = btp.tile([P, C, P], dt.bfloat16, tag="bt")
                nc.sync.dma_start(out=bt_t[:], in_=btv[:, t * C:(t + 1) * C, :])
                bts.append(bt_t)

            def issue_gathers(t):
                gcalls = []
                c0 = 0
                while c0 < C:
                    ch = min(GC, C - c0)
                    g = gp.tile([P, GC, F], dt.bfloat16, tag="g")
                    nidx = ch * P
                    w0 = t * W_IDX + c0 * (P // 16)
                    nc.gpsimd.dma_gather(
                        g[:, :ch, :], xb[:], idx_t[:, w0:w0 + nidx // 16],
                        nidx, nidx_regs[nidx], F, queue_num=qn[0] % NQ)
                    qn[0] += 1
                    gcalls.append((g, c0, ch))
                    c0 += ch
                return gcalls

            # all gathers issued up-front at max priority: keeps the 4 SWDGE
            # queues saturated for the whole kernel
            gcalls_per_tile = [issue_gathers(t) for t in range(NT)]

            for t in range(NT):
                gcalls = gcalls_per_tile[t]
                agg = aggps.tile([P, F], dt.float32, space="PSUM", tag="agg")
                for g, c0, ch in gcalls:
                    for j in range(ch):
                        c = c0 + j
                        nc.tensor.matmul(
                            agg[:], lhsT=bts[t][:, c, :], rhs=g[:, j, :],
                            start=(c == 0), stop=(c == C - 1))

                agg_sb = sbp.tile([P, F], dt.bfloat16, tag="aggsb")
                nc.vector.tensor_copy(out=agg_sb[:], in_=agg[:])

                aggT = sbp.tile([P, KT, P], dt.bfloat16, tag="aggT")
                for k in range(KT):
                    tp = trps.tile([P, P], dt.bfloat16, space="PSUM", tag="trp")
                    nc.tensor.transpose(
                        out=tp[:], in_=agg_sb[:, k * P:(k + 1) * P], identity=id_t[:])
                    nc.vector.tensor_copy(out=aggT[:, k, :], in_=tp[:])

                ops_ = outps.tile([P, H], dt.float32, space="PSUM", tag="ops")
                for k in range(KT):
                    nc.tensor.matmul(
                        ops_[:], lhsT=aggT[:, k, :], rhs=wt_t[:, k, :],
                        start=(k == 0), stop=(k == KT - 1 and not bias_nonzero))
                if bias_nonzero:
                    nc.tensor.matmul(
                        ops_[:], lhsT=ones_t[:1, :], rhs=bias_t[:1, :],
                        start=False, stop=True)

                # PReLU on the (otherwise idle) ScalarE: leaky-relu LUT with
                # the learned slope as alpha
                pos = sbp.tile([P, H], dt.float32, tag="pos")
                nc.scalar.activation(
                    out=pos[:], in_=ops_[:],
                    func=mybir.ActivationFunctionType.Prelu, alpha=float(alpha))

                valid = min(P, NPC - t * P)
                nc.sync.dma_start(out=outd[t * P:t * P + valid, :], in_=pos[:valid, :])

    nc.compile()
    return nc


def kernel(**inputs):
    from concourse.bass_utils import run_bass_kernel_spmd

    x = np.asarray(inputs["x"], dtype=np.float32)
    fc_w = np.asarray(inputs["fc_w"], dtype=np.float32)
    bias = np.asarray(inputs["bias"], dtype=np.float32).reshape(-1)
    alpha = float(np.asarray(inputs["prelu_a"]).reshape(-1)[0])
    adj_vals = np.asarray(inputs["adj_vals"], dtype=np.float32)
    adj_row = np.asarray(inputs["adj_row"]).astype(np.int64)
    adj_col = np.asarray(inputs["adj_col"]).astype(np.int64)

    # --- host-side 1D graph partition (nodes -> cores, tiles of 128) ---
    starts = []
    for i in range(NCORES):
        for t in range(NT):
            starts.append(i * NPC + min(t * P, NPC))
    starts.append(N)
    bounds = np.searchsorted(adj_row, np.asarray(starts))

    # Per (core, tile): deduplicate source columns (one gather per distinct
    # source, Bt column carries all its in-tile edges) and keep them sorted
    # for HBM locality.  tiles[j] = (uniq_cols, u_pos_per_edge, rows, vals)
    tiles = []
    n_uniq = np.zeros(NCORES * NT, np.int64)
    for j in range(NCORES * NT):
        lo, hi = int(bounds[j]), int(bounds[j + 1])
        cols = adj_col[lo:hi]
        uniq, inv = np.unique(cols, return_inverse=True)
        rl = adj_row[lo:hi] - starts[j]
        tiles.append((uniq, inv, rl, adj_vals[lo:hi]))
        n_uniq[j] = len(uniq)
    C = max(1, int(np.max((n_uniq + P - 1) // P)))  # uniform chunk budget per tile

    bias_nonzero = bool(np.any(bias != 0.0))
    key = (C, np.float32(alpha).tobytes(), bias_nonzero)
    if key not in _prog_cache:
        _prog_cache[key] = _build_program(C, alpha, bias_nonzero)
    nc = _prog_cache[key]

    W_IDX = C * P // 16
    xb = np.ascontiguousarray(x.astype(bf16))
    wtb = np.ascontiguousarray(fc_w.T.reshape(KT, P, H).transpose(1, 0, 2).astype(bf16))
    idn = np.eye(P, dtype=bf16)
    onesb = np.ones((1, P), dtype=bf16)
    biasb = bias.reshape(1, H).astype(bf16)

    in_maps = []
    for i in range(NCORES):
        gidx_flat = np.zeros(NT * C * P, np.int16)
        btv_f = np.zeros((P, NT * C, P), np.float32)
        for t in range(NT):
            j = i * NT + t
            uniq, inv, rl, vals = tiles[j]
            nu = len(uniq)
            if nu == 0:
                continue
            gidx_flat[t * C * P:t * C * P + nu] = uniq.astype(np.int16)
            np.add.at(btv_f, (inv % P, t * C + inv // P, rl), vals)
        # wrap indices: per GC-chunk gather call, column-major over 16 rows
        idx_host = np.zeros((P, NT * W_IDX), np.int16)
        for t in range(NT):
            c0 = 0
            while c0 < C:
                ch = min(GC, C - c0)
                nidx = ch * P
                seg = gidx_flat[t * C * P + c0 * P: t * C * P + c0 * P + nidx]
                w0 = t * W_IDX + c0 * (P // 16)
                idx_host[:, w0:w0 + nidx // 16] = np.tile(
                    seg.reshape(nidx // 16, 16).T, (8, 1))
                c0 += ch
        in_maps.append({
            "xb": xb, "gidx": idx_host, "btv": btv_f.astype(bf16),
            "wtb": wtb, "idn": idn, "onesb": onesb, "biasb": biasb,
        })

    res = run_bass_kernel_spmd(nc, in_maps, core_ids=list(range(NCORES)))
    out = np.concatenate([res.results[i]["out"] for i in range(NCORES)], axis=0)
    return np.ascontiguousarray(out.astype(np.float32))
